# revision 60
# baseline (speedup 1.0000x reference)
"""Causal multi-head attention (B=2, T=2048, C=1024, H=16) on 8 TRN2 NeuronCores.

Sharding: core = b*4 + g handles batch b and head-group g (4 heads, 256 of the
1024 channels). The host hands each core its batch's x pre-transposed (x^T,
[C, T]) plus its W column/row slices; everything on-device then runs in
"transposed activation" layout [feature, t] so the contraction dim always
sits on SBUF partitions:

  q^T/k^T = Wq/Wk slice (stationary) @ x^T (moving)      [m, t]
  v      = x^T (stationary) @ Wv slice (moving)          [t, m]  (+ ones col)
  S^T    = k^T_h.T-slice @ q^T_h   (K=64 contraction)    [j, i]  causal j<=i only
  E^T    = exp(S^T / 32) (ScalarE; diag causal mask via a PE-accumulated
           -1e6 upper-tri matmul before the exp)          [j, i]
  U^T+rs = V_aug (stationary) @ E^T  (ones col -> rowsum)[d+1, i]
  Y^T    = U^T * (1/rowsum) broadcast                     [d, i]
  O_part = Y^T (stationary) @ Wp slice (moving)           [t, n]

Matmuls run in float32r (full PE speed, ~1e-3 rel err); fp32->f32r rounding
happens in the SWDGE cast DMAs and PSUM-evacuation copies. Host sums the 4
head-group partials per batch and adds the bias.
"""
import numpy as np

import concourse.bass as bass
import concourse.mybir as mybir
import concourse.tile as tile
from concourse import bacc
from concourse.bass_utils import run_bass_kernel_spmd
from concourse.masks import make_identity, make_upper_triangular

F32 = mybir.dt.float32
F32R = mybir.dt.float32r
BF16 = mybir.dt.bfloat16
AF = mybir.ActivationFunctionType

B, T, C, H = 2, 2048, 1024, 16
D = C // H            # 64 head dim
HG = 4                # heads per core
CG = HG * D           # 256 channels per core
CC = C // 128         # 8 c-chunks
TC = T // 128         # 16 t-chunks
NHALF = T // 2
SCALE = C ** -0.5


def build_nc():
    nc = bacc.Bacc("TRN2", target_bir_lowering=False, debug=False)
    # host-pre-tiled layouts: xt[t4, p, cc, 512] == x^T tiled; w*[p, cc, m]
    xt = nc.dram_tensor("xt", [4, 128, CC, 512], F32, kind="ExternalInput")
    wq = nc.dram_tensor("wq", [128, CC, CG], F32, kind="ExternalInput")
    wk = nc.dram_tensor("wk", [128, CC, CG], F32, kind="ExternalInput")
    wv = nc.dram_tensor("wv", [128, CC, CG], F32, kind="ExternalInput")
    wp = nc.dram_tensor("wp", [128, 2, C], F32, kind="ExternalInput")
    o = nc.dram_tensor("o", [T, C], F32, kind="ExternalOutput")

    with tile.TileContext(nc) as tc:
        with (
            tc.tile_pool(name="const", bufs=1) as constp,
            tc.tile_pool(name="qkv", bufs=1) as qkvp,
            tc.tile_pool(name="proj", bufs=1) as projp,
            tc.tile_pool(name="wsl", bufs=1) as wslp,
            tc.tile_pool(name="xTp", bufs=1) as xTp,
            tc.tile_pool(name="att", bufs=2) as attp,
            tc.tile_pool(name="eP", bufs=4) as ep,
            tc.tile_pool(name="oout", bufs=4) as op_,
            # one PSUM pool, 8 banks: s (2 banks x2) score rows; u (1 bank
            # x2) U accumulators; w1 (1 bank x2) v/qk/output projections
            tc.tile_pool(name="psum", bufs=2, space="PSUM") as psp,
        ):
            identf = constp.tile([128, 128], F32, tag="identf")
            make_identity(nc, identf[:])
            identr = constp.tile([128, 128], BF16, tag="identr")
            nc.vector.tensor_copy(identr[:], identf[:])
            # strict upper-tri -BIG in [i, j]: accumulated onto the diagonal
            # score block via PE (out[j,i] = maskb[i,j]), masking j > i.
            # bf16 runs the PE at 1 cycle/row vs 4 for narrow f32r.
            maskbf = constp.tile([128, 128], F32, tag="maskbf")
            make_upper_triangular(nc, maskbf[:], val=-1e6, diag=False)
            maskb = constp.tile([128, 128], BF16, tag="maskb")
            nc.vector.tensor_copy(maskb[:], maskbf[:])
            onesc = constp.tile([128, 64], F32, tag="onesc")
            nc.gpsimd.memset(onesc[:], 1.0)

            qT = qkvp.tile([128, 2, T], F32R, tag="qT")
            kT = qkvp.tile([128, 2, T], F32R, tag="kT")
            vaug = qkvp.tile([128, TC, HG * (D + 1)], F32R, tag="vaug")
            wps = projp.tile([128, 2, C], F32R, tag="wps")
            yT = projp.tile([128, 2, T], F32R, tag="yT")
            wqs = wslp.tile([128, CC, CG], F32R, tag="wqs")
            wks = wslp.tile([128, CC, CG], F32R, tag="wks")
            wvs = wslp.tile([128, CC, CG], F32R, tag="wvs")
            xT = xTp.tile([128, CC, T], F32R, tag="xT")

            # ---- phase A/B emitter: x^T t-slice load + v/q/k projections ---
            def emit_ab_group(t4):
                if t4 == 0:
                    nc.gpsimd.dma_start(wvs[:], wv.ap())
                # x^T slice arrives via SWDGE cast DMA (fp32 -> f32r)
                nc.gpsimd.dma_start(
                    xT[:, :, t4 * 512:(t4 + 1) * 512], xt.ap()[t4])
                if t4 == 0:
                    for w_dram, w_sb in ((wq, wqs), (wk, wks)):
                        nc.gpsimd.dma_start(w_sb[:], w_dram.ap())
                for tl in range(4):
                    t_i = t4 * 4 + tl
                    # v for this t-chunk (natural layout)
                    psv = psp.tile([128, CG], F32, tag="w1")
                    for cc in range(CC):
                        nc.tensor.matmul(
                            psv[:], xT[:, cc, t_i * 128:(t_i + 1) * 128],
                            wvs[:, cc, :], start=(cc == 0), stop=(cc == CC - 1))
                    dstv = vaug[:, t_i, :].rearrange("p (h e) -> p h e", h=HG)
                    nc.vector.tensor_copy(
                        dstv[:, :, 0:D],
                        psv[:].rearrange("p (h e) -> p h e", e=D))

                # q^T / k^T for the 512-col t-group
                for w_sb, dstT in ((wqs, qT), (wks, kT)):
                    for mc in range(2):
                        psq = psp.tile([128, 512], F32, tag="w1")
                        for cc in range(CC):
                            nc.tensor.matmul(
                                psq[:], w_sb[:, cc, mc * 128:(mc + 1) * 128],
                                xT[:, cc, t4 * 512:(t4 + 1) * 512],
                                start=(cc == 0), stop=(cc == CC - 1))
                        nc.vector.tensor_copy(
                            dstT[:, mc, t4 * 512:(t4 + 1) * 512], psq[:])
                # ones column of v_aug for these 4 t-chunks
                nc.vector.tensor_copy(
                    vaug[:].rearrange(
                        "p t (h e) -> p t h e",
                        h=HG)[:, t4 * 4:(t4 + 1) * 4, :, D:D + 1],
                    onesc[:].rearrange(
                        "p (t h) -> p t h", h=HG)[:, t4 * 4:(t4 + 1) * 4, :, None])

            # ---- attention emitter: one head, one T-half ----
            def emit_head(half, h):
                i_lo = half * NHALF
                mc, pb = h // 2, (h % 2) * 64
                kTh = kT[pb:pb + 64, mc, :]
                qTh = qT[pb:pb + 64, mc, :]
                vh = vaug[:].rearrange("p t (h e) -> p t h e", h=HG)[:, :, h, :]
                chunks = (2 * half, 2 * half + 1)
                psu = {}
                for c in chunks:
                    psu_c = psp.tile([65, 512], F32, tag="u")
                    psu[c] = psu_c
                for j in range(8 * (half + 1)):
                    jt = j * 128
                    e0 = max(i_lo, jt)      # first valid i this row
                    # columns anchored at i_lo so each chunk's matmul stays
                    # inside one PSUM bank
                    pss = psp.tile([128, 1024], F32, tag="s")
                    for c in chunks:
                        if c < j // 4:
                            continue
                        i0 = max(c * 512, jt)
                        diag = i0 == jt and jt >= i_lo
                        nc.tensor.matmul(
                            pss[:, i0 - i_lo:(c + 1) * 512 - i_lo],
                            kTh[:, jt:jt + 128], qTh[:, i0:(c + 1) * 512],
                            start=True, stop=not diag)
                        if diag:  # causal mask via PE accumulate
                            nc.tensor.matmul(
                                pss[:, jt - i_lo:jt - i_lo + 128],
                                maskb[:], identr[:], start=False, stop=True)
                    eT = ep.tile([128, 1024], F32R, tag="e")
                    nc.scalar.activation(
                        eT[:, e0 - i_lo:NHALF], pss[:, e0 - i_lo:NHALF],
                        AF.Exp, scale=SCALE)
                    for c in chunks:
                        if c < j // 4:
                            continue
                        i0 = max(c * 512, jt)
                        nc.tensor.matmul(
                            psu[c][:, i0 - c * 512:512], vh[:, j, :],
                            eT[:, i0 - i_lo:(c + 1) * 512 - i_lo],
                            start=(j == 0), stop=(j == 4 * c + 3))
                        if j == 4 * c + 3:
                            # chunk done: softmax-normalize via the rowsum in
                            # row 64 of psu[c]
                            rs1 = attp.tile([1, 512], F32, tag="rs1")
                            nc.vector.tensor_copy(rs1[:], psu[c][64:65, :])
                            rin1 = attp.tile([1, 512], F32, tag="rin1")
                            nc.vector.reciprocal_approx_fast(rin1[:], rs1[:])
                            rinb = attp.tile([64, 512], F32, tag="rinb")
                            nc.gpsimd.partition_broadcast(
                                rinb[:], rin1[:], channels=64)
                            nc.vector.tensor_mul(
                                yT[pb:pb + 64, mc, c * 512:(c + 1) * 512],
                                psu[c][0:64, :], rinb[:])
                            if h == HG - 1:
                                emit_oproj(c)

            # ---- output projection for one finished 512-col chunk ----
            def emit_oproj(c):
                for t_i in range(4 * c, 4 * c + 4):
                    for nh in range(2):
                        pso = psp.tile([128, 512], F32, tag="w1")
                        for gc in range(2):
                            nc.tensor.matmul(
                                pso[:], yT[:, gc, t_i * 128:(t_i + 1) * 128],
                                wps[:, gc, nh * 512:(nh + 1) * 512],
                                start=(gc == 0), stop=(gc == 1))
                        oo = op_.tile([128, 512], F32, tag="oo")
                        if (t_i * 2 + nh) % 2:
                            nc.scalar.copy(oo[:], pso[:])
                        else:
                            nc.vector.tensor_copy(oo[:], pso[:])
                        nc.sync.dma_start(
                            o.ap()[t_i * 128:(t_i + 1) * 128,
                                   nh * 512:(nh + 1) * 512], oo[:])

            # ---- emission order: interleave projections with attention ----
            emit_ab_group(0)
            emit_ab_group(1)
            emit_head(0, 0)
            emit_head(0, 1)
            nc.gpsimd.dma_start(wps[:], wp.ap())
            emit_head(0, 2)
            emit_head(0, 3)
            emit_ab_group(2)
            emit_ab_group(3)
            for h in range(HG):
                emit_head(1, h)
    nc.compile()
    return nc


_NC_CACHE = {}


def _get_nc():
    if "nc" not in _NC_CACHE:
        _NC_CACHE["nc"] = build_nc()
    return _NC_CACHE["nc"]


def kernel(x, attention_mask, Wq, Wk, Wv, Wp, bp):
    x = np.asarray(x, np.float32)
    Wq = np.asarray(Wq, np.float32)
    Wk = np.asarray(Wk, np.float32)
    Wv = np.asarray(Wv, np.float32)
    Wp = np.asarray(Wp, np.float32)
    bp = np.asarray(bp, np.float32)
    del attention_mask  # all-ones; the reference's post-softmax masking is a no-op

    nc = _get_nc()
    # pre-tile to the SBUF layouts (pure data marshaling, no compute):
    # xt[t4, p, cc, 512] = x^T; w*[p, cc, m]; wp[p, gc, n]
    xts = [np.ascontiguousarray(
        x[b].T.reshape(CC, 128, 4, 512).transpose(2, 1, 0, 3))
        for b in range(B)]

    def wtile(w):  # [C, m] -> [128, CC, m]
        return np.ascontiguousarray(
            w.reshape(CC, 128, -1).transpose(1, 0, 2))

    in_maps = []
    for core in range(8):
        b, g = core // 4, core % 4
        cols = slice(g * CG, (g + 1) * CG)
        in_maps.append({
            "xt": xts[b],
            "wq": wtile(Wq[:, cols]),
            "wk": wtile(Wk[:, cols]),
            "wv": wtile(Wv[:, cols]),
            "wp": np.ascontiguousarray(
                Wp[cols, :].reshape(2, 128, C).transpose(1, 0, 2)),
        })
    res = run_bass_kernel_spmd(nc, in_maps, core_ids=list(range(8)))
    out = np.empty((B, T, C), np.float32)
    bp64 = bp.astype(np.float64)
    for b in range(B):
        acc = np.zeros((T, C), np.float64)
        for g in range(4):
            acc += res.results[b * 4 + g]["o"]
        out[b] = (acc + bp64).astype(np.float32)
    return out


# revision 61
# speedup vs baseline: 1.0744x; 1.0744x over previous
"""Causal multi-head attention (B=2, T=2048, C=1024, H=16) on 8 TRN2 NeuronCores.

Sharding: core = b*4 + g handles batch b and head-group g (4 heads, 256 of the
1024 channels). The host hands each core its batch's x pre-transposed (x^T,
[C, T]) plus its W column/row slices; everything on-device then runs in
"transposed activation" layout [feature, t] so the contraction dim always
sits on SBUF partitions:

  q^T/k^T = Wq/Wk slice (stationary) @ x^T (moving)      [m, t]
  v      = x^T (stationary) @ Wv slice (moving)          [t, m]  (+ ones col)
  S^T    = k^T_h.T-slice @ q^T_h   (K=64 contraction)    [j, i]  causal j<=i only
  E^T    = exp(S^T / 32) (ScalarE; diag causal mask via a PE-accumulated
           -1e6 upper-tri matmul before the exp)          [j, i]
  U^T+rs = V_aug (stationary) @ E^T  (ones col -> rowsum)[d+1, i]
  Y^T    = U^T * (1/rowsum) broadcast                     [d, i]
  O_part = Y^T (stationary) @ Wp slice (moving)           [t, n]

Matmuls run in float32r (full PE speed, ~1e-3 rel err); fp32->f32r rounding
happens in the SWDGE cast DMAs and PSUM-evacuation copies. Host sums the 4
head-group partials per batch and adds the bias.
"""
import numpy as np

import concourse.bass as bass
import concourse.mybir as mybir
import concourse.tile as tile
from concourse import bacc
from concourse.bass_utils import run_bass_kernel_spmd
from concourse.masks import make_identity, make_upper_triangular

F32 = mybir.dt.float32
F32R = mybir.dt.float32r
BF16 = mybir.dt.bfloat16
AF = mybir.ActivationFunctionType

B, T, C, H = 2, 2048, 1024, 16
D = C // H            # 64 head dim
HG = 4                # heads per core
CG = HG * D           # 256 channels per core
CC = C // 128         # 8 c-chunks
TC = T // 128         # 16 t-chunks
NHALF = T // 2
SCALE = C ** -0.5


def build_nc():
    nc = bacc.Bacc("TRN2", target_bir_lowering=False, debug=False)
    # host-pre-tiled layouts: xt[t4, p, cc, 512] == x^T tiled; w*[p, cc, m]
    xt = nc.dram_tensor("xt", [4, 128, CC, 512], F32, kind="ExternalInput")
    wq = nc.dram_tensor("wq", [128, CC, CG], F32, kind="ExternalInput")
    wk = nc.dram_tensor("wk", [128, CC, CG], F32, kind="ExternalInput")
    wv = nc.dram_tensor("wv", [128, CC, CG], F32, kind="ExternalInput")
    wp = nc.dram_tensor("wp", [128, 2, C], F32, kind="ExternalInput")
    o = nc.dram_tensor("o", [T, C], F32, kind="ExternalOutput")

    with tile.TileContext(nc) as tc:
        with (
            tc.tile_pool(name="const", bufs=1) as constp,
            tc.tile_pool(name="qkv", bufs=1) as qkvp,
            tc.tile_pool(name="proj", bufs=1) as projp,
            tc.tile_pool(name="wsl", bufs=1) as wslp,
            tc.tile_pool(name="xTp", bufs=1) as xTp,
            tc.tile_pool(name="att", bufs=2) as attp,
            tc.tile_pool(name="eP", bufs=4) as ep,
            tc.tile_pool(name="oout", bufs=4) as op_,
            # one PSUM pool, 8 banks: s (2 banks x2) score rows; u (1 bank
            # x2) U accumulators; w1 (1 bank x2) v/qk/output projections
            tc.tile_pool(name="psum", bufs=2, space="PSUM") as psp,
        ):
            identf = constp.tile([128, 128], F32, tag="identf")
            make_identity(nc, identf[:])
            identr = constp.tile([128, 128], BF16, tag="identr")
            nc.vector.tensor_copy(identr[:], identf[:])
            # strict upper-tri -BIG in [i, j]: accumulated onto the diagonal
            # score block via PE (out[j,i] = maskb[i,j]), masking j > i.
            # bf16 runs the PE at 1 cycle/row vs 4 for narrow f32r.
            maskbf = constp.tile([128, 128], F32, tag="maskbf")
            make_upper_triangular(nc, maskbf[:], val=-1e6, diag=False)
            maskb = constp.tile([128, 128], BF16, tag="maskb")
            nc.vector.tensor_copy(maskb[:], maskbf[:])
            onesc = constp.tile([128, 64], F32, tag="onesc")
            nc.gpsimd.memset(onesc[:], 1.0)

            qT = qkvp.tile([128, 2, T], F32R, tag="qT")
            kT = qkvp.tile([128, 2, T], F32R, tag="kT")
            vaug = qkvp.tile([128, TC, HG * (D + 1)], F32R, tag="vaug")
            wps = projp.tile([128, 2, C], F32R, tag="wps")
            yT = projp.tile([128, 2, T], F32R, tag="yT")
            wqs = wslp.tile([128, CC, CG], F32R, tag="wqs")
            wks = wslp.tile([128, CC, CG], F32R, tag="wks")
            wvs = wslp.tile([128, CC, CG], F32R, tag="wvs")
            xT = xTp.tile([128, CC, T], F32R, tag="xT")

            # ---- phase A/B emitter: x^T t-slice load + v/q/k projections ---
            def emit_ab_group(t4):
                if t4 == 0:
                    nc.gpsimd.dma_start(wvs[:], wv.ap())
                # x^T slice arrives via SWDGE cast DMA (fp32 -> f32r)
                nc.gpsimd.dma_start(
                    xT[:, :, t4 * 512:(t4 + 1) * 512], xt.ap()[t4])
                if t4 == 0:
                    for w_dram, w_sb in ((wq, wqs), (wk, wks)):
                        nc.gpsimd.dma_start(w_sb[:], w_dram.ap())
                for tl in range(4):
                    t_i = t4 * 4 + tl
                    # v for this t-chunk (natural layout)
                    psv = psp.tile([128, CG], F32, tag="w1")
                    for cc in range(CC):
                        nc.tensor.matmul(
                            psv[:], xT[:, cc, t_i * 128:(t_i + 1) * 128],
                            wvs[:, cc, :], start=(cc == 0), stop=(cc == CC - 1))
                    dstv = vaug[:, t_i, :].rearrange("p (h e) -> p h e", h=HG)
                    nc.vector.tensor_copy(
                        dstv[:, :, 0:D],
                        psv[:].rearrange("p (h e) -> p h e", e=D))

                # q^T / k^T for the 512-col t-group
                for w_sb, dstT in ((wqs, qT), (wks, kT)):
                    for mc in range(2):
                        psq = psp.tile([128, 512], F32, tag="w1")
                        for cc in range(CC):
                            nc.tensor.matmul(
                                psq[:], w_sb[:, cc, mc * 128:(mc + 1) * 128],
                                xT[:, cc, t4 * 512:(t4 + 1) * 512],
                                start=(cc == 0), stop=(cc == CC - 1))
                        nc.vector.tensor_copy(
                            dstT[:, mc, t4 * 512:(t4 + 1) * 512], psq[:])
                # ones column of v_aug for these 4 t-chunks
                nc.vector.tensor_copy(
                    vaug[:].rearrange(
                        "p t (h e) -> p t h e",
                        h=HG)[:, t4 * 4:(t4 + 1) * 4, :, D:D + 1],
                    onesc[:].rearrange(
                        "p (t h) -> p t h", h=HG)[:, t4 * 4:(t4 + 1) * 4, :, None])

            # ---- attention emitter: one head, one T-half ----
            def emit_head(half, h):
                i_lo = half * NHALF
                mc, pb = h // 2, (h % 2) * 64
                kTh = kT[pb:pb + 64, mc, :]
                qTh = qT[pb:pb + 64, mc, :]
                vh = vaug[:].rearrange("p t (h e) -> p t h e", h=HG)[:, :, h, :]
                chunks = (2 * half, 2 * half + 1)
                psu = {}
                for c in chunks:
                    psu_c = psp.tile([65, 512], F32, tag="u")
                    psu[c] = psu_c
                for j in range(8 * (half + 1)):
                    jt = j * 128
                    e0 = max(i_lo, jt)      # first valid i this row
                    # columns anchored at i_lo so each chunk's matmul stays
                    # inside one PSUM bank
                    pss = psp.tile([128, 1024], F32, tag="s")
                    for c in chunks:
                        if c < j // 4:
                            continue
                        i0 = max(c * 512, jt)
                        diag = i0 == jt and jt >= i_lo
                        nc.tensor.matmul(
                            pss[:, i0 - i_lo:(c + 1) * 512 - i_lo],
                            kTh[:, jt:jt + 128], qTh[:, i0:(c + 1) * 512],
                            start=True, stop=not diag)
                        if diag:  # causal mask via PE accumulate
                            nc.tensor.matmul(
                                pss[:, jt - i_lo:jt - i_lo + 128],
                                maskb[:], identr[:], start=False, stop=True)
                    eT = ep.tile([128, 1024], F32R, tag="e")
                    nc.scalar.activation(
                        eT[:, e0 - i_lo:NHALF], pss[:, e0 - i_lo:NHALF],
                        AF.Exp, scale=SCALE)
                    for c in chunks:
                        if c < j // 4:
                            continue
                        i0 = max(c * 512, jt)
                        nc.tensor.matmul(
                            psu[c][:, i0 - c * 512:512], vh[:, j, :],
                            eT[:, i0 - i_lo:(c + 1) * 512 - i_lo],
                            start=(j == 0), stop=(j == 4 * c + 3))
                        if j == 4 * c + 3:
                            # chunk done: softmax-normalize via the rowsum in
                            # row 64 of psu[c]
                            rs1 = attp.tile([1, 512], F32, tag="rs1")
                            nc.vector.tensor_copy(rs1[:], psu[c][64:65, :])
                            rin1 = attp.tile([1, 512], F32, tag="rin1")
                            nc.vector.reciprocal_approx_fast(rin1[:], rs1[:])
                            rinb = attp.tile([64, 512], F32, tag="rinb")
                            nc.gpsimd.partition_broadcast(
                                rinb[:], rin1[:], channels=64)
                            nc.vector.tensor_mul(
                                yT[pb:pb + 64, mc, c * 512:(c + 1) * 512],
                                psu[c][0:64, :], rinb[:])
                            if h == HG - 1:
                                emit_oproj(c)

            # ---- output projection for one finished 512-col chunk ----
            def emit_oproj(c):
                for t_i in range(4 * c, 4 * c + 4):
                    for nh in range(2):
                        pso = psp.tile([128, 512], F32, tag="w1")
                        for gc in range(2):
                            nc.tensor.matmul(
                                pso[:], yT[:, gc, t_i * 128:(t_i + 1) * 128],
                                wps[:, gc, nh * 512:(nh + 1) * 512],
                                start=(gc == 0), stop=(gc == 1))
                        oo = op_.tile([128, 512], F32, tag="oo")
                        if (t_i * 2 + nh) % 2:
                            nc.scalar.copy(oo[:], pso[:])
                        else:
                            nc.vector.tensor_copy(oo[:], pso[:])
                        nc.sync.dma_start(
                            o.ap()[t_i * 128:(t_i + 1) * 128,
                                   nh * 512:(nh + 1) * 512], oo[:])

            # ---- emission order: interleave projections with attention ----
            emit_ab_group(0)
            emit_ab_group(1)
            emit_head(0, 0)
            emit_head(0, 1)
            nc.gpsimd.dma_start(wps[:], wp.ap())
            emit_ab_group(2)
            emit_ab_group(3)
            emit_head(0, 2)
            emit_head(0, 3)
            for h in range(HG):
                emit_head(1, h)
    nc.compile()
    return nc


_NC_CACHE = {}


def _get_nc():
    if "nc" not in _NC_CACHE:
        _NC_CACHE["nc"] = build_nc()
    return _NC_CACHE["nc"]


def kernel(x, attention_mask, Wq, Wk, Wv, Wp, bp):
    x = np.asarray(x, np.float32)
    Wq = np.asarray(Wq, np.float32)
    Wk = np.asarray(Wk, np.float32)
    Wv = np.asarray(Wv, np.float32)
    Wp = np.asarray(Wp, np.float32)
    bp = np.asarray(bp, np.float32)
    del attention_mask  # all-ones; the reference's post-softmax masking is a no-op

    nc = _get_nc()
    # pre-tile to the SBUF layouts (pure data marshaling, no compute):
    # xt[t4, p, cc, 512] = x^T; w*[p, cc, m]; wp[p, gc, n]
    xts = [np.ascontiguousarray(
        x[b].T.reshape(CC, 128, 4, 512).transpose(2, 1, 0, 3))
        for b in range(B)]

    def wtile(w):  # [C, m] -> [128, CC, m]
        return np.ascontiguousarray(
            w.reshape(CC, 128, -1).transpose(1, 0, 2))

    in_maps = []
    for core in range(8):
        b, g = core // 4, core % 4
        cols = slice(g * CG, (g + 1) * CG)
        in_maps.append({
            "xt": xts[b],
            "wq": wtile(Wq[:, cols]),
            "wk": wtile(Wk[:, cols]),
            "wv": wtile(Wv[:, cols]),
            "wp": np.ascontiguousarray(
                Wp[cols, :].reshape(2, 128, C).transpose(1, 0, 2)),
        })
    res = run_bass_kernel_spmd(nc, in_maps, core_ids=list(range(8)))
    out = np.empty((B, T, C), np.float32)
    bp64 = bp.astype(np.float64)
    for b in range(B):
        acc = np.zeros((T, C), np.float64)
        for g in range(4):
            acc += res.results[b * 4 + g]["o"]
        out[b] = (acc + bp64).astype(np.float32)
    return out


# revision 63
# speedup vs baseline: 1.0757x; 1.0012x over previous
"""Causal multi-head attention (B=2, T=2048, C=1024, H=16) on 8 TRN2 NeuronCores.

Sharding: core = b*4 + g handles batch b and head-group g (4 heads, 256 of the
1024 channels). The host hands each core its batch's x pre-transposed (x^T,
[C, T]) plus its W column/row slices; everything on-device then runs in
"transposed activation" layout [feature, t] so the contraction dim always
sits on SBUF partitions:

  q^T/k^T = Wq/Wk slice (stationary) @ x^T (moving)      [m, t]
  v      = x^T (stationary) @ Wv slice (moving)          [t, m]  (+ ones col)
  S^T    = k^T_h.T-slice @ q^T_h   (K=64 contraction)    [j, i]  causal j<=i only
  E^T    = exp(S^T / 32) (ScalarE; diag causal mask via a PE-accumulated
           -1e6 upper-tri matmul before the exp)          [j, i]
  U^T+rs = V_aug (stationary) @ E^T  (ones col -> rowsum)[d+1, i]
  Y^T    = U^T * (1/rowsum) broadcast                     [d, i]
  O_part = Y^T (stationary) @ Wp slice (moving)           [t, n]

Matmuls run in float32r (full PE speed, ~1e-3 rel err); fp32->f32r rounding
happens in the SWDGE cast DMAs and PSUM-evacuation copies. Host sums the 4
head-group partials per batch and adds the bias.
"""
import numpy as np

import concourse.bass as bass
import concourse.mybir as mybir
import concourse.tile as tile
from concourse import bacc
from concourse.bass_utils import run_bass_kernel_spmd
from concourse.masks import make_identity, make_upper_triangular

F32 = mybir.dt.float32
F32R = mybir.dt.float32r
BF16 = mybir.dt.bfloat16
AF = mybir.ActivationFunctionType

B, T, C, H = 2, 2048, 1024, 16
D = C // H            # 64 head dim
HG = 4                # heads per core
CG = HG * D           # 256 channels per core
CC = C // 128         # 8 c-chunks
TC = T // 128         # 16 t-chunks
NHALF = T // 2
SCALE = C ** -0.5


def build_nc():
    nc = bacc.Bacc("TRN2", target_bir_lowering=False, debug=False)
    # host-pre-tiled layouts: xt[t4, p, cc, 512] == x^T tiled; w*[p, cc, m]
    xt = nc.dram_tensor("xt", [4, 128, CC, 512], F32, kind="ExternalInput")
    wq = nc.dram_tensor("wq", [128, CC, CG], F32, kind="ExternalInput")
    wk = nc.dram_tensor("wk", [128, CC, CG], F32, kind="ExternalInput")
    wv = nc.dram_tensor("wv", [128, CC, CG], F32, kind="ExternalInput")
    wp = nc.dram_tensor("wp", [128, 2, C], F32, kind="ExternalInput")
    o = nc.dram_tensor("o", [T, C], F32, kind="ExternalOutput")

    with tile.TileContext(nc) as tc:
        with (
            tc.tile_pool(name="const", bufs=1) as constp,
            tc.tile_pool(name="qkv", bufs=1) as qkvp,
            tc.tile_pool(name="proj", bufs=1) as projp,
            tc.tile_pool(name="wsl", bufs=1) as wslp,
            tc.tile_pool(name="xTp", bufs=1) as xTp,
            tc.tile_pool(name="att", bufs=2) as attp,
            tc.tile_pool(name="eP", bufs=5) as ep,
            tc.tile_pool(name="oout", bufs=4) as op_,
            # one PSUM pool, 8 banks: s (2 banks x2) score rows; u (1 bank
            # x2) U accumulators; w1 (1 bank x2) v/qk/output projections
            tc.tile_pool(name="psum", bufs=2, space="PSUM") as psp,
        ):
            identf = constp.tile([128, 128], F32, tag="identf")
            make_identity(nc, identf[:])
            identr = constp.tile([128, 128], BF16, tag="identr")
            nc.vector.tensor_copy(identr[:], identf[:])
            # strict upper-tri -BIG in [i, j]: accumulated onto the diagonal
            # score block via PE (out[j,i] = maskb[i,j]), masking j > i.
            # bf16 runs the PE at 1 cycle/row vs 4 for narrow f32r.
            maskbf = constp.tile([128, 128], F32, tag="maskbf")
            make_upper_triangular(nc, maskbf[:], val=-1e6, diag=False)
            maskb = constp.tile([128, 128], BF16, tag="maskb")
            nc.vector.tensor_copy(maskb[:], maskbf[:])
            onesc = constp.tile([128, 64], F32, tag="onesc")
            nc.gpsimd.memset(onesc[:], 1.0)

            qT = qkvp.tile([128, 2, T], F32R, tag="qT")
            kT = qkvp.tile([128, 2, T], F32R, tag="kT")
            vaug = qkvp.tile([128, TC, HG * (D + 1)], F32R, tag="vaug")
            wps = projp.tile([128, 2, C], F32R, tag="wps")
            yT = projp.tile([128, 2, T], F32R, tag="yT")
            wqs = wslp.tile([128, CC, CG], F32R, tag="wqs")
            wks = wslp.tile([128, CC, CG], F32R, tag="wks")
            wvs = wslp.tile([128, CC, CG], F32R, tag="wvs")
            xT = xTp.tile([128, CC, T], F32R, tag="xT")

            # ---- phase A/B emitter: x^T t-slice load + v/q/k projections ---
            def emit_ab_group(t4):
                if t4 == 0:
                    nc.gpsimd.dma_start(wvs[:], wv.ap())
                # x^T slice arrives via SWDGE cast DMA (fp32 -> f32r)
                nc.gpsimd.dma_start(
                    xT[:, :, t4 * 512:(t4 + 1) * 512], xt.ap()[t4])
                if t4 == 0:
                    for w_dram, w_sb in ((wq, wqs), (wk, wks)):
                        nc.gpsimd.dma_start(w_sb[:], w_dram.ap())
                for tl in range(4):
                    t_i = t4 * 4 + tl
                    # v for this t-chunk (natural layout)
                    psv = psp.tile([128, CG], F32, tag="w1")
                    for cc in range(CC):
                        nc.tensor.matmul(
                            psv[:], xT[:, cc, t_i * 128:(t_i + 1) * 128],
                            wvs[:, cc, :], start=(cc == 0), stop=(cc == CC - 1))
                    dstv = vaug[:, t_i, :].rearrange("p (h e) -> p h e", h=HG)
                    nc.vector.tensor_copy(
                        dstv[:, :, 0:D],
                        psv[:].rearrange("p (h e) -> p h e", e=D))

                # q^T / k^T for the 512-col t-group
                for w_sb, dstT in ((wqs, qT), (wks, kT)):
                    for mc in range(2):
                        psq = psp.tile([128, 512], F32, tag="w1")
                        for cc in range(CC):
                            nc.tensor.matmul(
                                psq[:], w_sb[:, cc, mc * 128:(mc + 1) * 128],
                                xT[:, cc, t4 * 512:(t4 + 1) * 512],
                                start=(cc == 0), stop=(cc == CC - 1))
                        nc.vector.tensor_copy(
                            dstT[:, mc, t4 * 512:(t4 + 1) * 512], psq[:])
                # ones column of v_aug for these 4 t-chunks
                nc.vector.tensor_copy(
                    vaug[:].rearrange(
                        "p t (h e) -> p t h e",
                        h=HG)[:, t4 * 4:(t4 + 1) * 4, :, D:D + 1],
                    onesc[:].rearrange(
                        "p (t h) -> p t h", h=HG)[:, t4 * 4:(t4 + 1) * 4, :, None])

            # ---- attention emitter: one head, one T-half ----
            def emit_head(half, h):
                i_lo = half * NHALF
                mc, pb = h // 2, (h % 2) * 64
                kTh = kT[pb:pb + 64, mc, :]
                qTh = qT[pb:pb + 64, mc, :]
                vh = vaug[:].rearrange("p t (h e) -> p t h e", h=HG)[:, :, h, :]
                chunks = (2 * half, 2 * half + 1)
                psu = {}
                for c in chunks:
                    psu_c = psp.tile([65, 512], F32, tag="u")
                    psu[c] = psu_c
                for j in range(8 * (half + 1)):
                    jt = j * 128
                    e0 = max(i_lo, jt)      # first valid i this row
                    # columns anchored at i_lo so each chunk's matmul stays
                    # inside one PSUM bank
                    pss = psp.tile([128, 1024], F32, tag="s")
                    for c in chunks:
                        if c < j // 4:
                            continue
                        i0 = max(c * 512, jt)
                        diag = i0 == jt and jt >= i_lo
                        nc.tensor.matmul(
                            pss[:, i0 - i_lo:(c + 1) * 512 - i_lo],
                            kTh[:, jt:jt + 128], qTh[:, i0:(c + 1) * 512],
                            start=True, stop=not diag)
                        if diag:  # causal mask via PE accumulate
                            nc.tensor.matmul(
                                pss[:, jt - i_lo:jt - i_lo + 128],
                                maskb[:], identr[:], start=False, stop=True)
                    eT = ep.tile([128, 1024], F32R, tag="e")
                    nc.scalar.activation(
                        eT[:, e0 - i_lo:NHALF], pss[:, e0 - i_lo:NHALF],
                        AF.Exp, scale=SCALE)
                    for c in chunks:
                        if c < j // 4:
                            continue
                        i0 = max(c * 512, jt)
                        nc.tensor.matmul(
                            psu[c][:, i0 - c * 512:512], vh[:, j, :],
                            eT[:, i0 - i_lo:(c + 1) * 512 - i_lo],
                            start=(j == 0), stop=(j == 4 * c + 3))
                        if j == 4 * c + 3:
                            # chunk done: softmax-normalize via the rowsum in
                            # row 64 of psu[c]
                            rs1 = attp.tile([1, 512], F32, tag="rs1")
                            nc.vector.tensor_copy(rs1[:], psu[c][64:65, :])
                            rin1 = attp.tile([1, 512], F32, tag="rin1")
                            nc.vector.reciprocal_approx_fast(rin1[:], rs1[:])
                            rinb = attp.tile([64, 512], F32, tag="rinb")
                            nc.gpsimd.partition_broadcast(
                                rinb[:], rin1[:], channels=64)
                            nc.vector.tensor_mul(
                                yT[pb:pb + 64, mc, c * 512:(c + 1) * 512],
                                psu[c][0:64, :], rinb[:])
                            if h == HG - 1:
                                emit_oproj(c)

            # ---- output projection for one finished 512-col chunk ----
            def emit_oproj(c):
                for t_i in range(4 * c, 4 * c + 4):
                    for nh in range(2):
                        pso = psp.tile([128, 512], F32, tag="w1")
                        for gc in range(2):
                            nc.tensor.matmul(
                                pso[:], yT[:, gc, t_i * 128:(t_i + 1) * 128],
                                wps[:, gc, nh * 512:(nh + 1) * 512],
                                start=(gc == 0), stop=(gc == 1))
                        oo = op_.tile([128, 512], F32, tag="oo")
                        if (t_i * 2 + nh) % 2:
                            nc.scalar.copy(oo[:], pso[:])
                        else:
                            nc.vector.tensor_copy(oo[:], pso[:])
                        nc.sync.dma_start(
                            o.ap()[t_i * 128:(t_i + 1) * 128,
                                   nh * 512:(nh + 1) * 512], oo[:])

            # ---- emission order: interleave projections with attention ----
            emit_ab_group(0)
            emit_ab_group(1)
            emit_head(0, 0)
            emit_head(0, 1)
            nc.gpsimd.dma_start(wps[:], wp.ap())
            emit_ab_group(2)
            emit_ab_group(3)
            emit_head(0, 2)
            emit_head(0, 3)
            for h in range(HG):
                emit_head(1, h)
    nc.compile()
    return nc


_NC_CACHE = {}


def _get_nc():
    if "nc" not in _NC_CACHE:
        _NC_CACHE["nc"] = build_nc()
    return _NC_CACHE["nc"]


def kernel(x, attention_mask, Wq, Wk, Wv, Wp, bp):
    x = np.asarray(x, np.float32)
    Wq = np.asarray(Wq, np.float32)
    Wk = np.asarray(Wk, np.float32)
    Wv = np.asarray(Wv, np.float32)
    Wp = np.asarray(Wp, np.float32)
    bp = np.asarray(bp, np.float32)
    del attention_mask  # all-ones; the reference's post-softmax masking is a no-op

    nc = _get_nc()
    # pre-tile to the SBUF layouts (pure data marshaling, no compute):
    # xt[t4, p, cc, 512] = x^T; w*[p, cc, m]; wp[p, gc, n]
    xts = [np.ascontiguousarray(
        x[b].T.reshape(CC, 128, 4, 512).transpose(2, 1, 0, 3))
        for b in range(B)]

    def wtile(w):  # [C, m] -> [128, CC, m]
        return np.ascontiguousarray(
            w.reshape(CC, 128, -1).transpose(1, 0, 2))

    in_maps = []
    for core in range(8):
        b, g = core // 4, core % 4
        cols = slice(g * CG, (g + 1) * CG)
        in_maps.append({
            "xt": xts[b],
            "wq": wtile(Wq[:, cols]),
            "wk": wtile(Wk[:, cols]),
            "wv": wtile(Wv[:, cols]),
            "wp": np.ascontiguousarray(
                Wp[cols, :].reshape(2, 128, C).transpose(1, 0, 2)),
        })
    res = run_bass_kernel_spmd(nc, in_maps, core_ids=list(range(8)))
    out = np.empty((B, T, C), np.float32)
    bp64 = bp.astype(np.float64)
    for b in range(B):
        acc = np.zeros((T, C), np.float64)
        for g in range(4):
            acc += res.results[b * 4 + g]["o"]
        out[b] = (acc + bp64).astype(np.float32)
    return out


# revision 64
# speedup vs baseline: 1.0968x; 1.0196x over previous
"""Causal multi-head attention (B=2, T=2048, C=1024, H=16) on 8 TRN2 NeuronCores.

Sharding: core = b*4 + g handles batch b and head-group g (4 heads, 256 of the
1024 channels). The host hands each core its batch's x pre-transposed (x^T,
[C, T]) plus its W column/row slices; everything on-device then runs in
"transposed activation" layout [feature, t] so the contraction dim always
sits on SBUF partitions:

  q^T/k^T = Wq/Wk slice (stationary) @ x^T (moving)      [m, t]
  v      = x^T (stationary) @ Wv slice (moving)          [t, m]  (+ ones col)
  S^T    = k^T_h.T-slice @ q^T_h   (K=64 contraction)    [j, i]  causal j<=i only
  E^T    = exp(S^T / 32) (ScalarE; diag causal mask via a PE-accumulated
           -1e6 upper-tri matmul before the exp)          [j, i]
  U^T+rs = V_aug (stationary) @ E^T  (ones col -> rowsum)[d+1, i]
  Y^T    = U^T * (1/rowsum) broadcast                     [d, i]
  O_part = Y^T (stationary) @ Wp slice (moving)           [t, n]

Matmuls run in float32r (full PE speed, ~1e-3 rel err); fp32->f32r rounding
happens in the SWDGE cast DMAs and PSUM-evacuation copies. Host sums the 4
head-group partials per batch and adds the bias.
"""
import numpy as np

import concourse.bass as bass
import concourse.mybir as mybir
import concourse.tile as tile
from concourse import bacc
from concourse.bass_utils import run_bass_kernel_spmd
from concourse.masks import make_identity, make_upper_triangular

F32 = mybir.dt.float32
F32R = mybir.dt.float32r
BF16 = mybir.dt.bfloat16
AF = mybir.ActivationFunctionType

B, T, C, H = 2, 2048, 1024, 16
D = C // H            # 64 head dim
HG = 4                # heads per core
CG = HG * D           # 256 channels per core
CC = C // 128         # 8 c-chunks
TC = T // 128         # 16 t-chunks
NHALF = T // 2
SCALE = C ** -0.5


def build_nc():
    nc = bacc.Bacc("TRN2", target_bir_lowering=False, debug=False)
    # host-pre-tiled layouts: xt[t4, p, cc, 512] == x^T tiled; w*[p, cc, m]
    xt = nc.dram_tensor("xt", [4, 128, CC, 512], F32, kind="ExternalInput")
    wq = nc.dram_tensor("wq", [128, CC, CG], F32, kind="ExternalInput")
    wk = nc.dram_tensor("wk", [128, CC, CG], F32, kind="ExternalInput")
    wv = nc.dram_tensor("wv", [128, CC, CG], F32, kind="ExternalInput")
    wp = nc.dram_tensor("wp", [128, 2, C], F32, kind="ExternalInput")
    o = nc.dram_tensor("o", [T, C], F32, kind="ExternalOutput")

    with tile.TileContext(nc) as tc:
        with (
            tc.tile_pool(name="const", bufs=1) as constp,
            tc.tile_pool(name="qkv", bufs=1) as qkvp,
            tc.tile_pool(name="proj", bufs=1) as projp,
            tc.tile_pool(name="wsl", bufs=1) as wslp,
            tc.tile_pool(name="xTp", bufs=1) as xTp,
            tc.tile_pool(name="att", bufs=2) as attp,
            tc.tile_pool(name="eP", bufs=5) as ep,
            tc.tile_pool(name="oout", bufs=6) as op_,
            # one PSUM pool, 8 banks: s (2 banks x2) score rows; u (1 bank
            # x2) U accumulators; w1 (1 bank x2) v/qk/output projections
            tc.tile_pool(name="psum", bufs=2, space="PSUM") as psp,
        ):
            identf = constp.tile([128, 128], F32, tag="identf")
            make_identity(nc, identf[:])
            identr = constp.tile([128, 128], BF16, tag="identr")
            nc.vector.tensor_copy(identr[:], identf[:])
            # strict upper-tri -BIG in [i, j]: accumulated onto the diagonal
            # score block via PE (out[j,i] = maskb[i,j]), masking j > i.
            # bf16 runs the PE at 1 cycle/row vs 4 for narrow f32r.
            maskbf = constp.tile([128, 128], F32, tag="maskbf")
            make_upper_triangular(nc, maskbf[:], val=-1e6, diag=False)
            maskb = constp.tile([128, 128], BF16, tag="maskb")
            nc.vector.tensor_copy(maskb[:], maskbf[:])
            onesc = constp.tile([128, 64], F32, tag="onesc")
            nc.gpsimd.memset(onesc[:], 1.0)

            qT = qkvp.tile([128, 2, T], F32R, tag="qT")
            kT = qkvp.tile([128, 2, T], F32R, tag="kT")
            vaug = qkvp.tile([128, TC, HG * (D + 1)], F32R, tag="vaug")
            wps = projp.tile([128, 2, C], F32R, tag="wps")
            yT = projp.tile([128, 2, T], F32R, tag="yT")
            wqs = wslp.tile([128, CC, CG], F32R, tag="wqs")
            wks = wslp.tile([128, CC, CG], F32R, tag="wks")
            wvs = wslp.tile([128, CC, CG], F32R, tag="wvs")
            xT = xTp.tile([128, CC, T], F32R, tag="xT")

            # ---- phase A/B emitter: x^T t-slice load + v/q/k projections ---
            def emit_ab_group(t4):
                if t4 == 0:
                    nc.gpsimd.dma_start(wvs[:], wv.ap())
                # x^T slice arrives via SWDGE cast DMA (fp32 -> f32r)
                nc.gpsimd.dma_start(
                    xT[:, :, t4 * 512:(t4 + 1) * 512], xt.ap()[t4])
                if t4 == 0:
                    for w_dram, w_sb in ((wq, wqs), (wk, wks)):
                        nc.gpsimd.dma_start(w_sb[:], w_dram.ap())
                for tl in range(4):
                    t_i = t4 * 4 + tl
                    # v for this t-chunk (natural layout)
                    psv = psp.tile([128, CG], F32, tag="w1")
                    for cc in range(CC):
                        nc.tensor.matmul(
                            psv[:], xT[:, cc, t_i * 128:(t_i + 1) * 128],
                            wvs[:, cc, :], start=(cc == 0), stop=(cc == CC - 1))
                    dstv = vaug[:, t_i, :].rearrange("p (h e) -> p h e", h=HG)
                    nc.vector.tensor_copy(
                        dstv[:, :, 0:D],
                        psv[:].rearrange("p (h e) -> p h e", e=D))

                # q^T / k^T for the 512-col t-group
                for w_sb, dstT in ((wqs, qT), (wks, kT)):
                    for mc in range(2):
                        psq = psp.tile([128, 512], F32, tag="w1")
                        for cc in range(CC):
                            nc.tensor.matmul(
                                psq[:], w_sb[:, cc, mc * 128:(mc + 1) * 128],
                                xT[:, cc, t4 * 512:(t4 + 1) * 512],
                                start=(cc == 0), stop=(cc == CC - 1))
                        nc.vector.tensor_copy(
                            dstT[:, mc, t4 * 512:(t4 + 1) * 512], psq[:])
                # ones column of v_aug for these 4 t-chunks
                nc.vector.tensor_copy(
                    vaug[:].rearrange(
                        "p t (h e) -> p t h e",
                        h=HG)[:, t4 * 4:(t4 + 1) * 4, :, D:D + 1],
                    onesc[:].rearrange(
                        "p (t h) -> p t h", h=HG)[:, t4 * 4:(t4 + 1) * 4, :, None])

            # ---- attention emitter: one head, one T-half ----
            def emit_head(half, h):
                i_lo = half * NHALF
                mc, pb = h // 2, (h % 2) * 64
                kTh = kT[pb:pb + 64, mc, :]
                qTh = qT[pb:pb + 64, mc, :]
                vh = vaug[:].rearrange("p t (h e) -> p t h e", h=HG)[:, :, h, :]
                chunks = (2 * half, 2 * half + 1)
                psu = {}
                for c in chunks:
                    psu_c = psp.tile([65, 512], F32, tag="u")
                    psu[c] = psu_c
                for j in range(8 * (half + 1)):
                    jt = j * 128
                    e0 = max(i_lo, jt)      # first valid i this row
                    # columns anchored at i_lo so each chunk's matmul stays
                    # inside one PSUM bank
                    pss = psp.tile([128, 1024], F32, tag="s")
                    for c in chunks:
                        if c < j // 4:
                            continue
                        i0 = max(c * 512, jt)
                        diag = i0 == jt and jt >= i_lo
                        nc.tensor.matmul(
                            pss[:, i0 - i_lo:(c + 1) * 512 - i_lo],
                            kTh[:, jt:jt + 128], qTh[:, i0:(c + 1) * 512],
                            start=True, stop=not diag)
                        if diag:  # causal mask via PE accumulate
                            nc.tensor.matmul(
                                pss[:, jt - i_lo:jt - i_lo + 128],
                                maskb[:], identr[:], start=False, stop=True)
                    eT = ep.tile([128, 1024], F32R, tag="e")
                    nc.scalar.activation(
                        eT[:, e0 - i_lo:NHALF], pss[:, e0 - i_lo:NHALF],
                        AF.Exp, scale=SCALE)
                    for c in chunks:
                        if c < j // 4:
                            continue
                        i0 = max(c * 512, jt)
                        nc.tensor.matmul(
                            psu[c][:, i0 - c * 512:512], vh[:, j, :],
                            eT[:, i0 - i_lo:(c + 1) * 512 - i_lo],
                            start=(j == 0), stop=(j == 4 * c + 3))
                        if j == 4 * c + 3:
                            # chunk done: softmax-normalize via the rowsum in
                            # row 64 of psu[c]
                            rs1 = attp.tile([1, 512], F32, tag="rs1")
                            nc.vector.tensor_copy(rs1[:], psu[c][64:65, :])
                            rin1 = attp.tile([1, 512], F32, tag="rin1")
                            nc.vector.reciprocal_approx_fast(rin1[:], rs1[:])
                            rinb = attp.tile([64, 512], F32, tag="rinb")
                            nc.gpsimd.partition_broadcast(
                                rinb[:], rin1[:], channels=64)
                            nc.vector.tensor_mul(
                                yT[pb:pb + 64, mc, c * 512:(c + 1) * 512],
                                psu[c][0:64, :], rinb[:])
                            if h == HG - 1:
                                emit_oproj(c)

            # ---- output projection for one finished 512-col chunk ----
            def emit_oproj(c):
                for t_i in range(4 * c, 4 * c + 4):
                    for nh in range(2):
                        pso = psp.tile([128, 512], F32, tag="w1")
                        for gc in range(2):
                            nc.tensor.matmul(
                                pso[:], yT[:, gc, t_i * 128:(t_i + 1) * 128],
                                wps[:, gc, nh * 512:(nh + 1) * 512],
                                start=(gc == 0), stop=(gc == 1))
                        oo = op_.tile([128, 512], F32, tag="oo")
                        if (t_i * 2 + nh) % 2:
                            nc.scalar.copy(oo[:], pso[:])
                        else:
                            nc.vector.tensor_copy(oo[:], pso[:])
                        nc.sync.dma_start(
                            o.ap()[t_i * 128:(t_i + 1) * 128,
                                   nh * 512:(nh + 1) * 512], oo[:])

            # ---- emission order: interleave projections with attention ----
            emit_ab_group(0)
            emit_ab_group(1)
            emit_head(0, 0)
            emit_head(0, 1)
            nc.gpsimd.dma_start(wps[:], wp.ap())
            emit_ab_group(2)
            emit_ab_group(3)
            emit_head(0, 2)
            emit_head(0, 3)
            for h in range(HG):
                emit_head(1, h)
    nc.compile()
    return nc


_NC_CACHE = {}


def _get_nc():
    if "nc" not in _NC_CACHE:
        _NC_CACHE["nc"] = build_nc()
    return _NC_CACHE["nc"]


def kernel(x, attention_mask, Wq, Wk, Wv, Wp, bp):
    x = np.asarray(x, np.float32)
    Wq = np.asarray(Wq, np.float32)
    Wk = np.asarray(Wk, np.float32)
    Wv = np.asarray(Wv, np.float32)
    Wp = np.asarray(Wp, np.float32)
    bp = np.asarray(bp, np.float32)
    del attention_mask  # all-ones; the reference's post-softmax masking is a no-op

    nc = _get_nc()
    # pre-tile to the SBUF layouts (pure data marshaling, no compute):
    # xt[t4, p, cc, 512] = x^T; w*[p, cc, m]; wp[p, gc, n]
    xts = [np.ascontiguousarray(
        x[b].T.reshape(CC, 128, 4, 512).transpose(2, 1, 0, 3))
        for b in range(B)]

    def wtile(w):  # [C, m] -> [128, CC, m]
        return np.ascontiguousarray(
            w.reshape(CC, 128, -1).transpose(1, 0, 2))

    in_maps = []
    for core in range(8):
        b, g = core // 4, core % 4
        cols = slice(g * CG, (g + 1) * CG)
        in_maps.append({
            "xt": xts[b],
            "wq": wtile(Wq[:, cols]),
            "wk": wtile(Wk[:, cols]),
            "wv": wtile(Wv[:, cols]),
            "wp": np.ascontiguousarray(
                Wp[cols, :].reshape(2, 128, C).transpose(1, 0, 2)),
        })
    res = run_bass_kernel_spmd(nc, in_maps, core_ids=list(range(8)))
    out = np.empty((B, T, C), np.float32)
    bp64 = bp.astype(np.float64)
    for b in range(B):
        acc = np.zeros((T, C), np.float64)
        for g in range(4):
            acc += res.results[b * 4 + g]["o"]
        out[b] = (acc + bp64).astype(np.float32)
    return out
